# revision 40
# baseline (speedup 1.0000x reference)
"""MultiHeadAttention (B=2, S=2048, D=1024, H=16) on 8 TRN2 NeuronCores.

Sharding: core c -> batch b = c//4, head-group g = c%4 (4 heads = 256 channels).
Each core computes its 4 heads' attention for its batch plus the partial
out-projection (out_w columns for its channel group); host sums the 4 partials
per batch and adds out_b.

Design (v5, ~220us vs 362us staged baseline; NTFF-trace-driven):
 - Exps batched: ONE ACTIVATE per key-chunk over a [128,1024] PSUM tile
   (2 banks, both heads side by side). ACT costs (N+352)/1.2 ns, so
   N=1024 halves the 352-cycle per-instruction overhead on what is the
   bottleneck engine at warm clock (measured 1112ns each).
 - Flat 128-iteration attention pipeline (pair-outer x qt x kc flattened):
   the next block's logits are emitted before the previous block's final
   AV matmuls (which wait on the last exp), so ACT never stalls at block
   boundaries. Each block's normalize is deferred into the next block's
   second slot. av PSUM tiles are allocated lazily inside the first AV
   emission so WAR deps link to the already-emitted d/avs copies.
 - Head is DMA-wire-bound (~150-190 GB/s effective with 8 cores streaming
   HBM concurrently; x alone is 4MB): all big inputs are host-packed into
   their exact SBUF layouts ([128, *] contiguous per partition), weights
   ride the gpsimd DMA queue, x chunks stripe sync/scalar queues, and the
   pre-attention projections run as a 6-accumulator dc-outer interleave
   (KT st0-3 + QT-qt0 + V chunk 0) consuming each x chunk as it lands, so
   the first exp fires ~1us after x finishes streaming.
 - Block (0,0) self-feeds the remaining V chunks just-in-time (chunk kc+1
   per slot); the remaining Q/K projections and the out-projection units
   stream into later kc slots as PE fill-in via generators (emission must
   always precede consumption -- deps are recorded at emission).
 - Denominators: ones column per head in Vaug (index HD in each VW=65
   group) -> denominators fall out of the AV matmul; [1,512] fp32 rows are
   reciprocal'd on DVE and partition-broadcast on the idle GpSimd engine
   (no PE matmuls). Final block's normalize is chunked with its
   out-projection interleaved over a 4-slot PSUM rotation and casts
   alternating DVE/ACT so the tail is matmul-paced.
 - PSUM budget (8 banks): lg (128,1024)x2 bufs + av0 + av1 (bufs=1, freed
   early via fp32 SBUF copies) + op x2.
 - Dtypes: all matmul operands 2-byte (fp16; exp outputs bf16 for range --
   logits reach ~50, e^50 overflows fp16), fp32 PSUM accumulation. fp32r
   draws the power throttle to its lowest p-state; fp8 runs at fp16 speed
   (duty-based clamp) and e4m3 quantization (~2.5% err) busts the 2e-2
   gate, so no fp8 anywhere.
 - Timing is thermally sensitive (several-us swings back-to-back).
"""

import os
import sys

import numpy as np

for _p in ("/opt/trn_rl_repo",):
    if os.path.isdir(_p) and _p not in sys.path:
        sys.path.insert(0, _p)

from collections import deque
from contextlib import ExitStack

import concourse.bass as bass
import concourse.tile as tile
from concourse import bacc, mybir
from concourse._compat import with_exitstack
from concourse.bass_utils import run_bass_kernel_spmd

B, S, D = 2, 2048, 1024
H = 16
HD = 64
NCORES = 8
JG = 256          # channels per core (4 heads)
DC = D // 128     # 8 contraction chunks
QT_TILES = 4      # 4 x 512 query tiles
KC = S // 128     # 16 key chunks
VW = 65           # V columns per head incl. ones column
FP32 = mybir.dt.float32
FP16 = mybir.dt.float16
BF16 = mybir.dt.bfloat16
EXP = mybir.ActivationFunctionType.Exp


@with_exitstack
def mha_core_kernel(ctx: ExitStack, tc: tile.TileContext,
                    out, xT, wqT, wkT, wvT, bq, bk, bv, owT):
    nc = tc.nc
    ctx.enter_context(nc.allow_low_precision("2-byte matmul operands"))

    persist = ctx.enter_context(tc.tile_pool(name="persist", bufs=1))
    QT_sb = persist.tile((128, 2 * S), FP16)
    KT_sb = persist.tile((128, 2 * S), FP16)
    Vaug_sb = persist.tile((128, KC * 4 * VW), BF16)
    attn_outT_sb = persist.tile((128, 2 * S), FP16)
    owT_sb = persist.tile((128, 2 * D), FP16)

    pA = ctx.enter_context(tc.tile_pool(name="pA", bufs=1))
    pB = ctx.enter_context(tc.tile_pool(name="pB", bufs=1))
    ps = ctx.enter_context(tc.tile_pool(name="ps", bufs=1, space="PSUM"))

    xT_sb = pA.tile((128, DC * S), FP16)
    wqT_sb = pA.tile((128, DC * JG), FP16)
    wkT_sb = pA.tile((128, DC * JG), FP16)
    wvT_sb = pA.tile((128, DC * JG), FP16)
    bq_sb = pA.tile((128, 2), FP32)
    bk_sb = pA.tile((128, 2), FP32)
    bv_bc = pA.tile((128, JG), FP32)
    ones_f32 = pA.tile((128, 64), FP32)

    # ---------------- DMA issues ----------------
    # All big tensors are HOST-PACKED into their exact SBUF layouts
    # ([128, *], one contiguous multi-KB run per partition): the strided
    # per-row descriptors were descriptor-bound (~20ns x 128 rows = 2.5us
    # per transfer regardless of size; x took ~28us to land). Weights on
    # the gpsimd queue, x chunks striped sync/scalar.
    nc.gpsimd.dma_start(out=wkT_sb[:, 0:DC * JG], in_=wkT[:, :])
    for st in range(QT_TILES):
        nc.sync.dma_start(
            out=xT_sb[:, st * 512:(st + 1) * 512],
            in_=xT[:, st * 512:(st + 1) * 512])
    xq = [nc.scalar, nc.sync]
    for dc in range(1, DC):
        xq[(dc - 1) % 2].dma_start(out=xT_sb[:, dc * S:(dc + 1) * S],
                                   in_=xT[:, dc * S:(dc + 1) * S])
    nc.gpsimd.dma_start(out=wqT_sb[:, 0:DC * JG], in_=wqT[:, :])
    nc.gpsimd.dma_start(out=wvT_sb[:, 0:DC * JG], in_=wvT[:, :])
    bq_ap = bass.AP(tensor=bq.tensor, offset=bq.offset,
                    ap=[[1, 128], [128, 2]])
    nc.gpsimd.dma_start(out=bq_sb[:, 0:2], in_=bq_ap)
    bk_ap = bass.AP(tensor=bk.tensor, offset=bk.offset,
                    ap=[[1, 128], [128, 2]])
    nc.gpsimd.dma_start(out=bk_sb[:, 0:2], in_=bk_ap)
    bv_bcast = bass.AP(tensor=bv.tensor, offset=bv.offset,
                       ap=[[0, 128]] + list(bv.ap))
    nc.gpsimd.dma_start(out=bv_bc, in_=bv_bcast)
    nc.gpsimd.dma_start(out=owT_sb[:, 0:2 * D], in_=owT[:, :])

    # ones: Vaug's per-head denominator columns + the K=1 broadcast row.
    # memset can't emit bf16-from-float cleanly everywhere; stage fp32 and
    # DVE-copy (converts) into the bf16 tiles. No DMA involved.
    nc.vector.memset(ones_f32, 1.0)
    nc.vector.tensor_copy(Vaug_sb[:, HD::VW], ones_f32)

    # ---------------- pre-attention projections ----------------
    def proj_unit(w_sb, b_sb, dst, jc, st):
        """Generator: one (weight, jc, st) projection chunk, 2 matmuls per
        next(), bias-add folded into the last step."""
        pu = ps.tile((128, 512), FP32, tag="op", bufs=2, name="pu")
        for dc in range(DC):
            nc.tensor.matmul(
                pu,
                w_sb[:, dc * JG + jc * 128:dc * JG + (jc + 1) * 128],
                xT_sb[:, dc * S + st * 512:dc * S + (st + 1) * 512],
                start=(dc == 0), stop=(dc == DC - 1),
            )
            if dc % 2 == 1 and dc < DC - 1:
                yield
        nc.vector.tensor_scalar_add(
            out=dst[:, jc * S + st * 512:jc * S + (st + 1) * 512],
            in0=pu, scalar1=b_sb[:, jc:jc + 1])
        yield

    # Pre-attention: dc-outer interleave of KT st0-3, QT-qt0 and V chunk 0
    # over 6 concurrent PSUM accumulators -- every consumer of x chunk dc
    # runs as soon as that chunk lands (the head is wire-bound, ~3us per
    # chunk), so the first exp can fire ~1us after x finishes streaming.
    pss = [ps.tile((128, 512), FP32, tag=["lg", "lg", "op", "op"][st],
                   bufs=2, name=f"kt{st}") for st in range(QT_TILES)]
    pq0 = ps.tile((128, 512), FP32, tag="av0", bufs=1, name="pq0")
    pv0 = ps.tile((128, 512), FP32, tag="av1", bufs=1, name="pv0")
    for dc in range(DC):
        for st in range(QT_TILES):
            nc.tensor.matmul(
                pss[st],
                wkT_sb[:, dc * JG:dc * JG + 128],
                xT_sb[:, dc * S + st * 512:dc * S + (st + 1) * 512],
                start=(dc == 0), stop=(dc == DC - 1),
            )
        nc.tensor.matmul(
            pq0,
            wqT_sb[:, dc * JG:dc * JG + 128],
            xT_sb[:, dc * S:dc * S + 512],
            start=(dc == 0), stop=(dc == DC - 1),
        )
        nc.tensor.matmul(
            pv0[:, 0:JG],
            xT_sb[:, dc * S:dc * S + 128],
            wvT_sb[:, dc * JG:(dc + 1) * JG],
            start=(dc == 0), stop=(dc == DC - 1),
        )
    for st in range(QT_TILES):
        nc.vector.tensor_scalar_add(
            out=KT_sb[:, st * 512:(st + 1) * 512],
            in0=pss[st], scalar1=bk_sb[:, 0:1])
    nc.vector.tensor_scalar_add(
        out=QT_sb[:, 0:512], in0=pq0, scalar1=bq_sb[:, 0:1])
    va0 = Vaug_sb[:, 0:4 * VW]
    nc.vector.tensor_add(
        out=bass.AP(tensor=va0.tensor, offset=va0.offset,
                    ap=[list(va0.ap[0]), [VW, 4], [1, HD]]),
        in0=bass.AP(tensor=pv0.tensor, offset=pv0.offset,
                    ap=[list(pv0.ap[0]), [HD, 4], [1, HD]]),
        in1=bass.AP(tensor=bv_bc.tensor, offset=bv_bc.offset,
                    ap=[list(bv_bc.ap[0]), [HD, 4], [1, HD]]))

    # V chunk: [s-chunk, j-local] into Vaug (stride 65), single strided
    # bias-add. Only chunk 0 runs pre-attention (inside the dc-outer
    # interleave above); block (0,0) self-feeds chunk kc+1 in its kc loop.
    def v_chunk(sc, tag):
        psv = ps.tile((128, 512), FP32, tag=tag,
                      bufs=1 if tag.startswith("av") else 2, name="psv")
        pv = psv[:, 0:JG]
        for dc in range(DC):
            nc.tensor.matmul(
                pv,
                xT_sb[:, dc * S + sc * 128:dc * S + (sc + 1) * 128],
                wvT_sb[:, dc * JG:(dc + 1) * JG],
                start=(dc == 0), stop=(dc == DC - 1),
            )
        base = sc * 4 * VW
        va = Vaug_sb[:, base:base + 4 * VW]
        nc.vector.tensor_add(
            out=bass.AP(tensor=va.tensor, offset=va.offset,
                        ap=[list(va.ap[0]), [VW, 4], [1, HD]]),
            in0=bass.AP(tensor=pv.tensor, offset=pv.offset,
                        ap=[list(pv.ap[0]), [HD, 4], [1, HD]]),
            in1=bass.AP(tensor=bv_bc.tensor, offset=bv_bc.offset,
                        ap=[list(bv_bc.ap[0]), [HD, 4], [1, HD]]))

    # Remaining projections stream into kc slots as PE fill-in. Emission
    # must always precede consumption (the tile framework records deps at
    # emission): QT-jc0-qt1 drains in block (0,0)'s last 4 slots, qt2/qt3
    # early in block (0,1), KT-jc1 well before pair 1.
    fill_q = deque()
    for st in range(2, QT_TILES):
        fill_q.append(proj_unit(wqT_sb, bq_sb, QT_sb, 0, st))
    for st in range(QT_TILES):
        fill_q.append(proj_unit(wkT_sb, bk_sb, KT_sb, 1, st))
    for st in range(QT_TILES):
        fill_q.append(proj_unit(wqT_sb, bq_sb, QT_sb, 1, st))

    def fill_step():
        while fill_q:
            try:
                next(fill_q[0])
                return
            except StopIteration:
                fill_q.popleft()

    def outproj_unit(st, it):
        """Generator: one [128,512] out-projection tile; 2 matmuls then
        cast+DMA, one next() each."""
        po = ps.tile((128, 512), FP32, tag="op", bufs=2, name="po")
        for jc in range(2):
            nc.tensor.matmul(
                po,
                attn_outT_sb[:, jc * S + st * 128:jc * S + st * 128 + 128],
                owT_sb[:, jc * D + it * 512:jc * D + (it + 1) * 512],
                start=(jc == 0), stop=(jc == 1))
        yield
        ost = pB.tile((128, 512), FP16, tag="ost", bufs=4, name="ost")
        nc.vector.tensor_copy(ost, po)
        nc.sync.dma_start(
            out=out[st * 128:(st + 1) * 128, it * 512:(it + 1) * 512],
            in_=ost)
        yield

    def queue_outproj(qt):
        for st in range(4 * qt, 4 * qt + 4):
            for it in range(2):
                fill_q.append(outproj_unit(st, it))

    # ---------------- attention (pair-outer) ----------------
    # Each block's normalize (bc matmuls + recip + muls) is DEFERRED into
    # the next block's first kc slot: emitting it at block end puts the bc
    # matmuls (which wait a ~1.5us DVE copy chain) ahead of the next
    # block's logits in the PE stream and stalls ACT ~2.5us per boundary.
    def make_normalize(avs0, avs1, d0, d1, base, chunked=False):
        def emit():
            # reciprocal of the [1,512] denominator rows on DVE (full-tile
            # base-0 APs as reciprocal_approx_fast requires), then a
            # partition-stride-0 DMA broadcast on the idle gpsimd queue --
            # replaces two K=1 PE matmuls per block (~3.4us of PE total).
            r0 = pB.tile((1, 512), FP32, tag="r0", bufs=2, name="r0")
            nc.vector.reciprocal_approx_fast(r0, d0)
            r1 = pB.tile((1, 512), FP32, tag="r1", bufs=2, name="r1")
            nc.vector.reciprocal_approx_fast(r1, d1)
            rcs0 = pB.tile((128, 512), FP32, tag="rcs", bufs=2, name="rcs0")
            nc.gpsimd.partition_broadcast(rcs0, r0)
            rcs1 = pB.tile((128, 512), FP32, tag="rcs", bufs=2, name="rcs1")
            nc.gpsimd.partition_broadcast(rcs1, r1)
            chunks = ((0, 128, (12,)), (128, 512, (13, 14, 15))) if chunked \
                else ((0, 512, ()),)
            nu = 0
            for lo, hi, sts in chunks:
                nc.vector.tensor_mul(
                    out=attn_outT_sb[0:HD, base + lo:base + hi],
                    in0=avs0[0:HD, lo:hi], in1=rcs0[0:HD, lo:hi])
                nc.vector.tensor_mul(
                    out=attn_outT_sb[HD:128, base + lo:base + hi],
                    in0=avs1[0:HD, lo:hi], in1=rcs1[0:HD, lo:hi])
                # tail out-projection: rotate over 4 PSUM slots (the lg
                # banks are free once the exps are done) and alternate
                # casts between DVE and the now-idle ACT so the tail is
                # matmul-paced, not cast-paced.
                for st in sts:
                    for it in range(2):
                        po = ps.tile((128, 512), FP32,
                                     tag=["op", "lg"][nu % 2], bufs=2,
                                     name="pof")
                        for jc in range(2):
                            nc.tensor.matmul(
                                po,
                                attn_outT_sb[:, jc * S + st * 128:
                                             jc * S + st * 128 + 128],
                                owT_sb[:, jc * D + it * 512:
                                       jc * D + (it + 1) * 512],
                                start=(jc == 0), stop=(jc == 1))
                        ost = pB.tile((128, 512), FP16, tag="ost", bufs=4,
                                      name="ost")
                        if nu % 2 == 0:
                            nc.vector.tensor_copy(ost, po)
                        else:
                            nc.scalar.activation(
                                ost, po, mybir.ActivationFunctionType.Copy)
                        nc.sync.dma_start(
                            out=out[st * 128:(st + 1) * 128,
                                    it * 512:(it + 1) * 512],
                            in_=ost)
                        nu += 1
        return emit

    # Flat 128-iteration pipeline: the next block's logits are emitted
    # BEFORE the previous block's last AV matmuls (which wait on the last
    # exp), so ACT never stalls at block boundaries (~0.9us x 15 saved).
    st8 = {"av0": None, "av1": None, "norm": None}

    def emit_av(kc, at, pair):
        if kc == 0:
            # lazy alloc: must come after the PREVIOUS block's av readers
            # (the d/avs copies) are emitted, or the WAR is mis-linked
            st8["av0"] = ps.tile((128, 512), FP32, tag="av0", bufs=1,
                                 name="av0")
            st8["av1"] = ps.tile((128, 512), FP32, tag="av1", bufs=1,
                                 name="av1")
        for h, avp, off in ((2 * pair, st8["av0"], 0),
                            (2 * pair + 1, st8["av1"], 512)):
            nc.tensor.matmul(
                avp[0:VW, :],
                Vaug_sb[:, kc * 4 * VW + h * VW:
                        kc * 4 * VW + (h + 1) * VW],
                at[:, off:off + 512],
                start=(kc == 0), stop=(kc == KC - 1))

    def finish_block(pair, qt):
        # denominator rows first (the normalize needs them soonest), then
        # the av rows to SBUF fp32 -- frees the av banks so the next
        # block's first AV matmul doesn't WAR-wait the normalize.
        av0, av1 = st8["av0"], st8["av1"]
        d0 = pB.tile((1, 512), FP32, tag="d0", bufs=2, name="d0")
        nc.vector.tensor_copy(d0, av0[HD:HD + 1, :])
        d1 = pB.tile((1, 512), FP32, tag="d1", bufs=2, name="d1")
        nc.vector.tensor_copy(d1, av1[HD:HD + 1, :])
        avs0 = pB.tile((VW, 512), FP32, tag="avs0", bufs=2, name="avs0")
        nc.vector.tensor_copy(avs0, av0[0:VW, :])
        avs1 = pB.tile((VW, 512), FP32, tag="avs1", bufs=2, name="avs1")
        nc.vector.tensor_copy(avs1, av1[0:VW, :])
        st8["norm"] = make_normalize(
            avs0, avs1, d0, d1, pair * S + qt * 512,
            chunked=(pair == 1 and qt == QT_TILES - 1))

    pend = None
    for gi in range(2 * QT_TILES * KC):
        b, kc = divmod(gi, KC)
        pair, qt = divmod(b, QT_TILES)
        lg = ps.tile((128, 1024), FP32, tag="lg", bufs=2, name="lg")
        kcol = pair * S + kc * 128
        qcol = pair * S + qt * 512
        nc.tensor.matmul(
            lg[:, 0:512],
            KT_sb[0:64, kcol:kcol + 128],
            QT_sb[0:64, qcol:qcol + 512],
            start=True, stop=True, tile_position=(0, 0))
        nc.tensor.matmul(
            lg[:, 512:1024],
            KT_sb[64:128, kcol:kcol + 128],
            QT_sb[64:128, qcol:qcol + 512],
            start=True, stop=True, tile_position=(64, 0))
        if b == 0:
            # block (0,0) self-feeds: V chunk kc+1 just-in-time for the
            # next iteration's AV (chunk 0 was pre-made). QT-jc0-qt1 runs
            # whole in the last slot -- a fill generator here would
            # interleave its PSUM accumulation with the V chunks' op-tag
            # rotation and get clobbered.
            if kc < KC - 1:
                v_chunk(kc + 1, "op")
            else:
                for _ in proj_unit(wqT_sb, bq_sb, QT_sb, 0, 1):
                    pass
        else:
            fill_step()
        if pend is not None:
            emit_av(pend[0], pend[1], pend[2])
            if pend[0] == KC - 1:
                finish_block(pend[2], pend[3])
        at = pB.tile((128, 1024), BF16, tag="at", bufs=3, name="at")
        nc.scalar.activation(at, lg, EXP)
        pend = (kc, at, pair, qt)
        if kc == 1:
            if st8["norm"] is not None:
                st8["norm"]()
                st8["norm"] = None
            if pair == 1 and qt > 0:
                queue_outproj(qt - 1)

    # tail: the final block's AV + normalize (with its out-projection
    # st-units interleaved), after draining leftover fill units
    emit_av(pend[0], pend[1], pend[2])
    finish_block(pend[2], pend[3])
    while fill_q:
        fill_step()
    st8["norm"]()


_NC = None


def _build_nc():
    global _NC
    if _NC is not None:
        return _NC
    nc = bacc.Bacc("TRN2", target_bir_lowering=False, debug=False,
                   num_devices=NCORES)
    # all big inputs host-packed to SBUF layout [128, *]
    xT = nc.dram_tensor("xT", [128, DC * S], FP16, kind="ExternalInput").ap()
    wqT = nc.dram_tensor("wqT", [128, DC * JG], FP16,
                         kind="ExternalInput").ap()
    wkT = nc.dram_tensor("wkT", [128, DC * JG], FP16,
                         kind="ExternalInput").ap()
    wvT = nc.dram_tensor("wvT", [128, DC * JG], FP16,
                         kind="ExternalInput").ap()
    bq = nc.dram_tensor("bq", [JG], FP32, kind="ExternalInput").ap()
    bk = nc.dram_tensor("bk", [JG], FP32, kind="ExternalInput").ap()
    bv = nc.dram_tensor("bv", [JG], FP32, kind="ExternalInput").ap()
    owT = nc.dram_tensor("owT", [128, 2 * D], FP16, kind="ExternalInput").ap()
    out = nc.dram_tensor("out", [S, D], FP16, kind="ExternalOutput").ap()
    with tile.TileContext(nc) as tc:
        mha_core_kernel(tc, out, xT, wqT, wkT, wvT, bq, bk, bv, owT)
    nc.compile()
    _NC = nc
    return nc


def _pack(a):
    """[n*128, m] -> [128, n*m]: row p holds chunks (p, 128+p, ...) side by
    side -- the SBUF-resident layout, so each partition's DMA is one
    contiguous run."""
    n = a.shape[0] // 128
    return np.ascontiguousarray(
        a.reshape(n, 128, a.shape[1]).transpose(1, 0, 2).reshape(128, -1))


def _in_maps(x, kqv_w, kqv_b, out_w):
    maps = []
    xP = [_pack(x[b].T.astype(np.float16)) for b in range(B)]
    for c in range(NCORES):
        b, g = divmod(c, 4)
        sl = slice(g * JG, (g + 1) * JG)
        maps.append({
            "xT": xP[b],
            "wqT": _pack(kqv_w[0 * D:1 * D][sl].T.astype(np.float16)),
            "wkT": _pack(kqv_w[1 * D:2 * D][sl].T.astype(np.float16)),
            "wvT": _pack(kqv_w[2 * D:3 * D][sl].T.astype(np.float16)),
            "bq": np.ascontiguousarray(kqv_b[0 * D:1 * D][sl]),
            "bk": np.ascontiguousarray(kqv_b[1 * D:2 * D][sl]),
            "bv": np.ascontiguousarray(kqv_b[2 * D:3 * D][sl]),
            "owT": _pack(out_w[:, sl].T.astype(np.float16)),
        })
    return maps


def run_spmd(x, kqv_w, kqv_b, out_w, out_b, trace=False, tmpdir=None):
    nc = _build_nc()
    res = run_bass_kernel_spmd(nc, _in_maps(x, kqv_w, kqv_b, out_w),
                               list(range(NCORES)), tmpdir=tmpdir, trace=trace)
    parts = [np.asarray(res.results[c]["out"], dtype=np.float32)
             for c in range(NCORES)]
    full = np.stack([
        parts[4 * b] + parts[4 * b + 1] + parts[4 * b + 2] + parts[4 * b + 3]
        + out_b[None, :].astype(np.float32)
        for b in range(B)
    ])
    return full, res


def kernel(**inputs):
    x = np.asarray(inputs["x"], dtype=np.float32)
    kqv_w = np.asarray(inputs["kqv_w"], dtype=np.float32)
    kqv_b = np.asarray(inputs["kqv_b"], dtype=np.float32)
    out_w = np.asarray(inputs["out_w"], dtype=np.float32)
    out_b = np.asarray(inputs["out_b"], dtype=np.float32)
    full, _ = run_spmd(x, kqv_w, kqv_b, out_w, out_b)
    return full
